# revision 26
# baseline (speedup 1.0000x reference)
"""Trainium2 Bass kernel for CascadeCodebookCluster (vq_codebook).

Contract: kernel(**inputs) takes FULL unsharded inputs (embeds [8,2048,256],
cb0 [5832,256], cb1 [324,256], cb2 [18,256], all fp32) and returns
(q_stack [3,8,2048,256] fp32, q_loss scalar fp32), matching reference().

Strategy (data-parallel over the flattened token dim: 8 cores, one batch row
per core; codebooks replicated on every core):
  - Host splits embeddings E^T and codebooks W^T into fp16 hi/lo pairs
    (e = e_hi + e_lo to ~22 mantissa bits). The PE computes the biased score
      s[n,k] = e_n.w_k - 0.5*||w_k||^2
    per 128-token tile in 14 chunks of <=512 entries via 7 fp16 matmuls per
    chunk accumulated in PSUM: hi.hi + hi.lo + lo.hi (3-pass fp16 == fp32-tier
    precision, 3x faster than native fp32 matmul) plus a K=2 matmul of a ones
    vector against (-bias_hi, -bias_lo) rows, which applies the per-entry bias
    with no extra DVE work.  argmax_k s == argmin_k ||e - w_k||^2 with
    first-occurrence tie-breaking.
  - ScalarE (ACT) copies each PSUM chunk into the SBUF score tile; VectorE
    does max8 + max_index per codebook region (first-occurrence argmax) and
    tiny index/max bookkeeping copies.
  - GPSIMD indirect DMA gathers the winning codebook rows straight from DRAM
    (per-partition row indices), and one DMA per layer stores the q shard.
  - Loss finishes on host: d_min = ||e||^2 - 2*s_max per token/layer;
    q_loss = 1.25 * sum_l mean(d_min_l) (commitment + codebook MSE).
"""

import numpy as np

B, S, D = 8, 2048, 256
TOK = 2048              # tokens per core
NT = TOK // 128         # 16 token tiles per core
K0, K1, K2 = 5832, 324, 18
K0P, K1P, K2P = 6144, 512, 32
KTOT = K0P + K1P + K2P  # 6688
# (col0, width) of the 14 matmul chunks (12 cb0 chunks, cb1, cb2)
CHUNKS = [(c * 512, 512) for c in range(13)] + [(13 * 512, 32)]
# per codebook: (region col0, region width)
REGIONS = [(0, K0P), (K0P, K1P), (K0P + K1P, K2P)]
PAD_BIAS = 60000.0      # fp16-exact, dwarfs any real score
N_CORES = 8
# chunks 0..N_SUB-1 apply the bias on DVE (tensor_sub from PSUM, fp32 bias);
# chunks N_SUB.. apply it as a 7th fp16 K=2 matmul + ScalarE copy.  N_SUB=0
# measured fastest: routing PSUM drains through the (busy) DVE stalls the PE.
N_SUB = 0


def _build_nc(stage=3):
    """Build the per-core Bass program (SPMD: identical on all 8 cores)."""
    import concourse.bass as bass
    import concourse.mybir as mybir
    from concourse import bacc, tile

    f32 = mybir.dt.float32
    f16 = mybir.dt.float16
    u32 = mybir.dt.uint32

    nc = bacc.Bacc(None, target_bir_lowering=False, debug=False)

    # ---- DRAM I/O ----
    eh_d = [nc.dram_tensor(f"eh{h}", [128, TOK], f16, kind="ExternalInput")
            for h in range(2)]
    el_d = [nc.dram_tensor(f"el{h}", [128, TOK], f16, kind="ExternalInput")
            for h in range(2)]
    wh_d = [nc.dram_tensor(f"wh{h}", [128, KTOT], f16, kind="ExternalInput")
            for h in range(2)]
    wl_d = [nc.dram_tensor(f"wl{h}", [128, KTOT], f16, kind="ExternalInput")
            for h in range(2)]
    bias_d = nc.dram_tensor("bias2", [2, KTOT], f16, kind="ExternalInput")
    bias32_d = None
    if N_SUB:
        bias32_d = nc.dram_tensor(
            "bias32", [128, 512 * N_SUB], f32, kind="ExternalInput"
        )
    cbg_d = [
        nc.dram_tensor("cbg0", [K0, D], f32, kind="ExternalInput"),
        nc.dram_tensor("cbg1", [K1, D], f32, kind="ExternalInput"),
        nc.dram_tensor("cbg2", [K2, D], f32, kind="ExternalInput"),
    ]
    q_d = [
        nc.dram_tensor(f"q{l}", [TOK, D], f32, kind="ExternalOutput")
        for l in range(3)
    ]
    mout_d = nc.dram_tensor("mout", [128, 3 * NT], f32, kind="ExternalOutput")

    with tile.TileContext(nc) as tc:
        with (
            tc.tile_pool(name="const", bufs=1) as constp,
            tc.tile_pool(name="spool", bufs=2) as spool,
            tc.tile_pool(name="tilep", bufs=2) as tilep,
            tc.tile_pool(name="psum", bufs=7, space="PSUM") as psump,
            tc.tile_pool(name="idxp", bufs=3 * NT) as idxp,
            tc.tile_pool(name="qpool", bufs=1) as qpool,
        ):
            # ---- resident loads ----
            ehs = [constp.tile([128, TOK], f16, name=f"ehs{h}") for h in range(2)]
            els = [constp.tile([128, TOK], f16, name=f"els{h}") for h in range(2)]
            whs = [constp.tile([128, KTOT], f16, name=f"whs{h}") for h in range(2)]
            wls = [constp.tile([128, KTOT], f16, name=f"wls{h}") for h in range(2)]
            bias2s = constp.tile([2, KTOT], f16)
            ones2 = constp.tile([2, 128], f16)
            nc.vector.memset(ones2[:, :], 1.0)
            for h in range(2):
                nc.sync.dma_start(ehs[h][:, :], eh_d[h][:, :])
                nc.sync.dma_start(els[h][:, :], el_d[h][:, :])
            # W loads split along k so the first chunk groups can start
            # before the whole codebook has landed
            WPIECES = [(0, 2048), (2048, 2048), (4096, 2592)]
            for (pc0, pw) in WPIECES:
                for h in range(2):
                    nc.sync.dma_start(
                        whs[h][:, pc0 : pc0 + pw], wh_d[h][:, pc0 : pc0 + pw]
                    )
                    nc.sync.dma_start(
                        wls[h][:, pc0 : pc0 + pw], wl_d[h][:, pc0 : pc0 + pw]
                    )
            nc.sync.dma_start(bias2s[:, :], bias_d[:, :])
            if N_SUB:
                bias32s = constp.tile([128, 512 * N_SUB], f32)
                nc.sync.dma_start(bias32s[:, :], bias32_d[:, :])

            # per-layer per-tile max scores, for host-side loss
            mall = constp.tile([128, 3, NT], f32)
            # q gather buffers, one per layer, filled tile by tile
            qbs = [
                qpool.tile([128, NT, D], f32, name=f"qb{l}") for l in range(3)
            ]

            # chunk-groups of at most 4 PSUM banks, uniform kind per group
            GROUPS = [list(range(4)), list(range(4, 8)),
                      list(range(8, 12)), list(range(12, 14))]

            for t in range(NT):
                s_t = spool.tile([128, KTOT], f32, name="s_t")
                m8 = tilep.tile([128, 3, 8], f32, name="m8")
                i8 = tilep.tile([128, 3, 8], u32, name="i8")

                tok = slice(t * 128, (t + 1) * 128)
                for grp in GROUPS:
                    pss = {}
                    for c in grp:
                        pss[c] = psump.tile([128, 512], f32, name="ps")
                    # accumulating passes: hh0 hh1 hl0 hl1 lh0 lh1 [bias]
                    passes = [
                        (ehs[0][:, tok], whs[0]), (ehs[1][:, tok], whs[1]),
                        (ehs[0][:, tok], wls[0]), (ehs[1][:, tok], wls[1]),
                        (els[0][:, tok], whs[0]), (els[1][:, tok], whs[1]),
                    ]
                    if grp[0] >= N_SUB:
                        passes.append((ones2[:, :], bias2s))
                    for pi, (lhs, rhs) in enumerate(passes):
                        for c in grp:
                            col0, w = CHUNKS[c]
                            nc.tensor.matmul(
                                pss[c][:, :w], lhs, rhs[:, col0 : col0 + w],
                                start=(pi == 0), stop=(pi == len(passes) - 1),
                            )
                    for c in grp:
                        col0, w = CHUNKS[c]
                        if c < N_SUB:
                            # bias applied on DVE; lands in SBUF directly
                            nc.vector.tensor_sub(
                                s_t[:, col0 : col0 + w], pss[c][:, :w],
                                bias32s[:, col0 : col0 + w],
                            )
                        else:
                            # bias already accumulated by the PE; ACT copies
                            nc.scalar.copy(
                                s_t[:, col0 : col0 + w], pss[c][:, :w]
                            )

                for l, (col0, w) in enumerate(REGIONS):
                    # top-8 values then their (first-occurrence) indices
                    nc.vector.max(m8[:, l], s_t[:, col0 : col0 + w])
                    nc.vector.max_index(
                        i8[:, l], m8[:, l], s_t[:, col0 : col0 + w]
                    )
                    # bookkeeping copies ride on ScalarE, not the busy DVE
                    nc.scalar.copy(mall[:, l, t : t + 1], m8[:, l, 0:1])
                    if stage < 3:
                        continue
                    # gather the winning codebook rows right away so the
                    # indirect DMAs overlap with later tiles' compute
                    idxc = idxp.tile([128, 1], u32, name="idxc")
                    nc.scalar.copy(idxc[:, :], i8[:, l, 0:1])
                    nc.gpsimd.indirect_dma_start(
                        out=qbs[l][:, t, :],
                        out_offset=None,
                        in_=cbg_d[l][:, :],
                        in_offset=bass.IndirectOffsetOnAxis(
                            ap=idxc[:, :], axis=0
                        ),
                    )

            nc.sync.dma_start(mout_d[:, :], mall[:, :, :])
            if stage >= 3:
                for l in range(3):
                    nc.sync.dma_start(
                        q_d[l].rearrange("(t p) d -> p t d", p=128),
                        qbs[l][:, :, :],
                    )

    return nc


_NC_CACHE = {}


def _get_nc():
    if "nc" not in _NC_CACHE:
        import os
        stage = int(os.environ.get("KERNEL_STAGE", "3"))
        nc = _build_nc(stage)
        nc.finalize()  # Bacc.finalize runs compile() (incl. alloc_regs)
        _NC_CACHE["nc"] = nc
    return _NC_CACHE["nc"]


def _hilo(x):
    hi = x.astype(np.float16)
    lo = (x - hi.astype(np.float32)).astype(np.float16)
    return np.ascontiguousarray(hi), np.ascontiguousarray(lo)


def _host_prep(embeds, cb0, cb1, cb2):
    """Build per-core input maps (host-side sharding + fp16 hi/lo splits)."""
    embeds = np.ascontiguousarray(np.asarray(embeds, dtype=np.float32))
    cbs = [np.ascontiguousarray(np.asarray(c, dtype=np.float32))
           for c in (cb0, cb1, cb2)]

    # W^T concatenated along k with zero padding, split in d-halves
    wt = np.zeros((D, KTOT), dtype=np.float32)
    wt[:, 0:K0] = cbs[0].T
    wt[:, K0P : K0P + K1] = cbs[1].T
    wt[:, K0P + K1P : K0P + K1P + K2] = cbs[2].T
    w_hi, w_lo = _hilo(wt)

    # negated bias rows: s = e.w + (-0.5*||w||^2); padded entries get a
    # huge negative score so they never win the argmax
    nbias = np.full((KTOT,), -PAD_BIAS, dtype=np.float64)
    nbias[0:K0] = -0.5 * np.sum(cbs[0].astype(np.float64) ** 2, axis=1)
    nbias[K0P : K0P + K1] = -0.5 * np.sum(cbs[1].astype(np.float64) ** 2, axis=1)
    nbias[K0P + K1P : K0P + K1P + K2] = (
        -0.5 * np.sum(cbs[2].astype(np.float64) ** 2, axis=1)
    )
    b_hi = nbias.astype(np.float16)
    b_lo = (nbias - b_hi.astype(np.float64)).astype(np.float16)
    bias2 = np.ascontiguousarray(np.stack([b_hi, b_lo], axis=0))

    common = {
        "wh0": w_hi[:128], "wh1": w_hi[128:],
        "wl0": w_lo[:128], "wl1": w_lo[128:],
        "bias2": bias2,
        "cbg0": cbs[0], "cbg1": cbs[1], "cbg2": cbs[2],
    }
    if N_SUB:
        # positive fp32 bias for the DVE-subtracted chunks, replicated
        # across partitions
        common["bias32"] = np.ascontiguousarray(np.broadcast_to(
            (-nbias[: 512 * N_SUB]).astype(np.float32)[None, :],
            (128, 512 * N_SUB),
        ))
    in_maps = []
    for c in range(N_CORES):
        et = embeds[c].T  # [256, 2048] fp32
        e_hi, e_lo = _hilo(np.ascontiguousarray(et))
        in_maps.append({
            "eh0": e_hi[:128], "eh1": e_hi[128:],
            "el0": e_lo[:128], "el1": e_lo[128:],
            **common,
        })
    return embeds, cbs, in_maps


def _host_post(embeds, results):
    """Assemble q_stack and the scalar loss from per-core outputs."""
    q_stack = np.empty((3, B, S, D), dtype=np.float32)
    loss = 0.0
    n_elems = float(B * S * D)
    for c in range(N_CORES):
        r = results[c]
        for l in range(3):
            q_stack[l, c] = r[f"q{l}"].reshape(S, D)
        # mout[p, l*NT + t] = s_max of token t*128+p for layer l
        m = r["mout"].reshape(128, 3, NT).astype(np.float64)
        esq = np.sum(embeds[c].astype(np.float64) ** 2, axis=-1)  # [2048]
        esq_pt = esq.reshape(NT, 128).T  # [p, t]
        for l in range(3):
            d_min = esq_pt - 2.0 * m[:, l, :]
            loss += 1.25 * d_min.sum() / n_elems
    return q_stack, np.float32(loss)


def kernel(embeds, cb0, cb1, cb2):
    from concourse.bass_utils import run_bass_kernel_spmd

    embeds, cbs, in_maps = _host_prep(embeds, cb0, cb1, cb2)
    nc = _get_nc()
    import os
    trace = bool(int(os.environ.get("KERNEL_TRACE", "0")))
    res = run_bass_kernel_spmd(
        nc, in_maps, core_ids=list(range(N_CORES)), trace=trace
    )
    _NC_CACHE["last_results"] = res
    return _host_post(embeds, res.results)
